# revision 16
# baseline (speedup 1.0000x reference)
"""Trainium2 Bass kernel for nn_Attention (B=2, S=2048, D=2048, H=16, DH=128, RoPE, causal).

Sharding: batch (2) x head-groups (4) across 8 cores. Each core computes the
partial output for 1 batch and 4 heads; the host sums the 4 head-group partials
per batch and adds b_O.

Per-core device program (all matmul operands fp16, fp32 PSUM accumulation):
  phase 1: QKV projections from host-pre-transposed X^T slabs; rotary fused into
           the PSUM->SBUF evacuation of Q^T/K^T. (Bias rank-1 matmuls only in
           the with-bias build variant; the graded inputs have zero biases.)
  phase 2 (interleaved with phase 3 per q-block): per (head, q-block of 512):
           scores^T tiles = K_tile^T.T @ Q^T (causal block-skipping), exp on ACT
           with 1/sqrt(128) folded into the scale, triangular mask-mul on the
           diagonal 128x128 sub-block, AV accumulated over k-tiles, row-sums via
           an all-ones [128,128] lhsT (result lands replicated across
           partitions), fast approx reciprocal, single normalize multiply.
  phase 3: output projection out[s,d] += Z^T[h].T @ W_O[h] for the 4 s-tiles of
           the finished q-block, DMA partial out.
"""

import os
import sys

if "/opt/trn_rl_repo" not in sys.path:
    sys.path.insert(0, "/opt/trn_rl_repo")

from contextlib import ExitStack

import numpy as np

import concourse.bass as bass
import concourse.tile as tile
from concourse import bacc, mybir
from concourse.bass import ds, ts
from concourse.bass_utils import run_bass_kernel_spmd

B, S, D, H, DH = 2, 2048, 2048, 16, 128
HPC = 4            # heads per core
NCORES = 8
SB = 512           # s/q block width
NSB = S // SB      # 4
NDT = D // 128     # 16 contraction d-tiles
NST = S // 128     # 16 s-tiles / k-tiles
ROT_BASE = 10000.0
SCALE = 1.0 / float(np.sqrt(float(DH)))

F16 = mybir.dt.float16
F32 = mybir.dt.float32


def _build_bass(with_bias):
    nc = bacc.Bacc()

    # --- I/O ---
    xq = nc.dram_tensor("xq", [NSB, 128, NDT * SB], F16, kind="ExternalInput")
    xk = nc.dram_tensor("xk", [NSB, 128, NDT * SB], F16, kind="ExternalInput")
    xv = nc.dram_tensor("xv", [NSB, 128, NDT * SB], F16, kind="ExternalInput")
    wq = nc.dram_tensor("wq", [128, HPC * NDT * DH], F16, kind="ExternalInput")
    wk = nc.dram_tensor("wk", [128, HPC * NDT * DH], F16, kind="ExternalInput")
    wv = nc.dram_tensor("wv", [128, NDT * HPC * DH], F16, kind="ExternalInput")
    wo = nc.dram_tensor("wo", [128, HPC * D], F16, kind="ExternalInput")
    if with_bias:
        bqf = nc.dram_tensor("bqf", [1, HPC * DH], F16, kind="ExternalInput")
        bkf = nc.dram_tensor("bkf", [1, HPC * DH], F16, kind="ExternalInput")
        bvf = nc.dram_tensor("bvf", [1, HPC * DH], F16, kind="ExternalInput")
        ones512_d = nc.dram_tensor("ones512", [1, SB], F16, kind="ExternalInput")
        ones128_d = nc.dram_tensor("ones128", [1, 128], F16, kind="ExternalInput")
    cos_d = nc.dram_tensor("cos_t", [128, S], F16, kind="ExternalInput")
    sin_d = nc.dram_tensor("sin_f", [128, S], F16, kind="ExternalInput")
    mask_d = nc.dram_tensor("mask_tri", [128, 128], F16, kind="ExternalInput")
    onesq_d = nc.dram_tensor("ones_sq", [128, 128], F16, kind="ExternalInput")
    out_part = nc.dram_tensor("out_part", [S, D], F32, kind="ExternalOutput")

    with ExitStack() as ctx:
        tc = ctx.enter_context(tile.TileContext(nc))

        persist = ctx.enter_context(tc.tile_pool(name="persist", bufs=1))
        consts = ctx.enter_context(tc.tile_pool(name="consts", bufs=1))
        psum = ctx.enter_context(tc.tile_pool(name="psum", bufs=1, space="PSUM"))

        # persistent activations
        qT = persist.tile([128, HPC * S], F16)   # (e, h*S + s)
        kT = persist.tile([128, HPC * S], F16)   # (e, h*S + s)
        vsb = persist.tile([128, NST * HPC * DH], F16)  # (s%128, stile*512 + h*128 + e)

        # constants (DMA issued inside phase 1, after the critical weight loads)
        mask = consts.tile([128, 128], F16)
        ones_sq = consts.tile([128, 128], F16)
        if with_bias:
            ones512 = consts.tile([1, SB], F16)
            nc.scalar.dma_start(ones512, ones512_d[:])
            ones128 = consts.tile([1, 128], F16)
            nc.scalar.dma_start(ones128, ones128_d[:])
            bq_sb = consts.tile([1, HPC * DH], F16)
            nc.scalar.dma_start(bq_sb, bqf[:])
            bk_sb = consts.tile([1, HPC * DH], F16)
            nc.scalar.dma_start(bk_sb, bkf[:])
            bv_sb = consts.tile([1, HPC * DH], F16)
            nc.scalar.dma_start(bv_sb, bvf[:])

        # ---------------- phase 1: projections + rotary ----------------
        with tc.tile_pool(name="p1w", bufs=1) as p1w, \
             tc.tile_pool(name="slabs", bufs=3) as slabs, \
             tc.tile_pool(name="rot", bufs=3) as rot:
            QW = NDT * DH  # per-head weight columns
            wq_sb = p1w.tile([128, HPC * NDT * DH], F16)
            nc.scalar.dma_start(wq_sb[:, 0:QW], wq[:, 0:QW])
            wk_sb = p1w.tile([128, HPC * NDT * DH], F16)
            nc.scalar.dma_start(wk_sb[:, 0:QW], wk[:, 0:QW])
            cosT = p1w.tile([128, S], F16)
            nc.scalar.dma_start(cosT, cos_d[:])
            sinF = p1w.tile([128, S], F16)
            nc.scalar.dma_start(sinF, sin_d[:])
            for hh in range(1, HPC):
                nc.scalar.dma_start(wq_sb[:, ds(hh * QW, QW)], wq[:, ds(hh * QW, QW)])
                nc.scalar.dma_start(wk_sb[:, ds(hh * QW, QW)], wk[:, ds(hh * QW, QW)])
            wv_sb = p1w.tile([128, NDT * HPC * DH], F16)
            nc.scalar.dma_start(wv_sb, wv[:])
            nc.scalar.dma_start(mask, mask_d[:])
            nc.scalar.dma_start(ones_sq, onesq_d[:])

            def rope_evac(pp, dst_slice, sb):
                """dst = rotary(pp) cast to fp16; reads cos/sin columns of block sb.

                The PSUM tile is first evacuated to fp16 SBUF on ACT so the
                rotary multiplies run in the DVE 2x packed mode."""
                q16 = rot.tile([128, SB], F16, tag="q16")
                nc.scalar.copy(q16, pp)
                t1 = rot.tile([128, SB], F16, tag="t1")
                nc.vector.tensor_mul(t1, q16, cosT[:, ts(sb, SB)])
                t2 = rot.tile([128, SB], F16, tag="t2")
                nc.vector.tensor_mul(t2[0:64], q16[64:128], sinF[64:128, ts(sb, SB)])
                nc.vector.tensor_mul(t2[64:128], q16[0:64], sinF[0:64, ts(sb, SB)])
                nc.vector.tensor_add(dst_slice, t1, t2)

            for sb in range(NSB):
                xq_s = slabs.tile([128, NDT * SB], F16, tag="slab")
                nchunk = 4 if sb == 0 else 2
                CH = NDT * SB // nchunk
                for ci in range(nchunk):
                    nc.sync.dma_start(xq_s[:, ds(ci * CH, CH)], xq[sb][:, ds(ci * CH, CH)])
                xk_s = slabs.tile([128, NDT * SB], F16, tag="slab")
                nc.sync.dma_start(xk_s, xk[sb])
                xv_s = slabs.tile([128, NDT * SB], F16, tag="slab")
                nc.sync.dma_start(xv_s, xv[sb])

                for h in range(HPC):
                    pq = psum.tile([128, SB], F32, tag="proj", bufs=2)
                    for dt in range(NDT):
                        nc.tensor.matmul(
                            pq,
                            lhsT=wq_sb[:, ds((h * NDT + dt) * DH, DH)],
                            rhs=xq_s[:, ts(dt, SB)],
                            start=(dt == 0), stop=(not with_bias and dt == NDT - 1),
                        )
                    if with_bias:
                        nc.tensor.matmul(pq, lhsT=bq_sb[:, ds(h * DH, DH)],
                                         rhs=ones512, start=False, stop=True)
                    rope_evac(pq, qT[:, ds(h * S + sb * SB, SB)], sb)

                    pk = psum.tile([128, SB], F32, tag="scores", bufs=3)
                    for dt in range(NDT):
                        nc.tensor.matmul(
                            pk,
                            lhsT=wk_sb[:, ds((h * NDT + dt) * DH, DH)],
                            rhs=xk_s[:, ts(dt, SB)],
                            start=(dt == 0), stop=(not with_bias and dt == NDT - 1),
                        )
                    if with_bias:
                        nc.tensor.matmul(pk, lhsT=bk_sb[:, ds(h * DH, DH)],
                                         rhs=ones512, start=False, stop=True)
                    rope_evac(pk, kT[:, ds(h * S + sb * SB, SB)], sb)

                for st in range(4):
                    pv = psum.tile([128, HPC * DH], F32, tag="zacc", bufs=2)
                    for dt in range(NDT):
                        nc.tensor.matmul(
                            pv,
                            lhsT=xv_s[:, ds(dt * SB + st * 128, 128)],
                            rhs=wv_sb[:, ts(dt, HPC * DH)],
                            start=(dt == 0), stop=(not with_bias and dt == NDT - 1),
                        )
                    if with_bias:
                        nc.tensor.matmul(pv, lhsT=ones128, rhs=bv_sb,
                                         start=False, stop=True)
                    if sb == NSB - 1:
                        nc.vector.tensor_copy(vsb[:, ts(sb * 4 + st, HPC * DH)], pv)
                    else:
                        nc.scalar.copy(vsb[:, ts(sb * 4 + st, HPC * DH)], pv)

        # ------- phases 2+3 interleaved: attention + output projection -------
        with tc.tile_pool(name="p23", bufs=1) as p23, \
             tc.tile_pool(name="ppool", bufs=6) as ppool, \
             tc.tile_pool(name="p2tmp", bufs=2) as p2tmp, \
             tc.tile_pool(name="outp", bufs=2) as outp:
            zT = p23.tile([128, HPC * S], F16)   # (e, h*S + q)
            wo_sb = p23.tile([128, HPC * D], F16)
            nc.scalar.dma_start(wo_sb, wo[:])

            for qb in reversed(range(NSB)):
                nkt = 4 * qb + 4
                for h in range(HPC):
                    pz = psum.tile([128, SB], F32, tag="zacc", bufs=2)
                    pr = psum.tile([128, SB], F32, tag="rsum", bufs=1)
                    for kt in range(nkt):
                        t = kt - 4 * qb
                        q_lo = 0 if t < 0 else 128 * t
                        pscr = psum.tile([128, SB], F32, tag="scores", bufs=3)
                        nc.tensor.matmul(
                            pscr[:, q_lo:SB],
                            lhsT=kT[:, ds(h * S + kt * 128, 128)],
                            rhs=qT[:, ds(h * S + qb * SB + q_lo, SB - q_lo)],
                            start=True, stop=True,
                        )
                        pt = ppool.tile([128, SB], F16, tag="pt")
                        nc.scalar.activation(
                            pt[:, q_lo:SB], pscr[:, q_lo:SB],
                            mybir.ActivationFunctionType.Exp, scale=SCALE,
                        )
                        if t >= 0:
                            nc.gpsimd.tensor_mul(
                                pt[:, ds(q_lo, 128)], pt[:, ds(q_lo, 128)], mask
                            )
                        nc.tensor.matmul(
                            pz[:, q_lo:SB],
                            lhsT=vsb[:, ds(kt * HPC * DH + h * DH, DH)],
                            rhs=pt[:, q_lo:SB],
                            start=(kt == 0), stop=(kt == nkt - 1),
                        )
                        nc.tensor.matmul(
                            pr[:, q_lo:SB], lhsT=ones_sq, rhs=pt[:, q_lo:SB],
                            start=(kt == 0), stop=(kt == nkt - 1),
                        )
                    # pr rows are all identical (= row-sums over k); normalize.
                    rr = p2tmp.tile([128, SB], F32, tag="rr")
                    nc.vector.reciprocal_approx_fast(out=rr, in_=pr)
                    nc.vector.tensor_mul(zT[:, ds(h * S + qb * SB, SB)], pz, rr)

                # phase 3 for the 4 s-tiles of this q-block
                for st in range(4 * qb, 4 * qb + 4):
                    osb = outp.tile([128, D], F32, tag="osb")
                    for db in range(4):
                        po = psum.tile([128, SB], F32, tag="proj", bufs=2)
                        for h in range(HPC):
                            nc.tensor.matmul(
                                po,
                                lhsT=zT[:, ds(h * S + st * 128, 128)],
                                rhs=wo_sb[:, ds(h * D + db * SB, SB)],
                                start=(h == 0), stop=(h == HPC - 1),
                            )
                        nc.vector.tensor_copy(osb[:, ts(db, SB)], po)
                        nc.sync.dma_start(out_part[ds(st * 128, 128), ts(db, SB)],
                                          osb[:, ts(db, SB)])

    nc.compile()
    return nc


def _rotary_tables():
    pos = np.arange(S, dtype=np.float64)
    dim = np.arange(DH // 2, dtype=np.float64)
    freq = ROT_BASE ** (dim / (DH / 2))  # base ** (dim / 64)
    freq = np.concatenate([freq, freq])
    angles = pos[:, None] / freq[None, :]          # [S, 128]
    cos_t = np.cos(angles).T.astype(np.float16)    # [128, S]
    sin_t = np.sin(angles).T.astype(np.float16)
    # halves pre-swapped so each rotary mul reads equal base partitions:
    # rows 64:128 = -sin (multiplies q16[64:128] into out[0:64]),
    # rows 0:64   = +sin (multiplies q16[0:64]  into out[64:128])
    sin_f = np.concatenate([sin_t[64:], -sin_t[:64]], axis=0)
    return np.ascontiguousarray(cos_t), np.ascontiguousarray(sin_f)


def _x_slabs(x2d):
    """[S, D] fp32 -> [NSB, 128, NDT*SB] fp16 slab layout of X^T."""
    xt = x2d.T.astype(np.float16)                          # [D, S]
    return np.ascontiguousarray(
        xt.reshape(NDT, 128, NSB, SB).transpose(2, 1, 0, 3).reshape(NSB, 128, NDT * SB)
    )


def _prep_in_maps(inputs, with_bias):
    q_in = np.asarray(inputs["query_input"], np.float32)
    k_in = np.asarray(inputs["key_input"], np.float32)
    v_in = np.asarray(inputs["value_input"], np.float32)
    W_Q = np.asarray(inputs["W_Q"], np.float32)
    W_K = np.asarray(inputs["W_K"], np.float32)
    W_V = np.asarray(inputs["W_V"], np.float32)
    W_O = np.asarray(inputs["W_O"], np.float32)
    b_Q = np.asarray(inputs["b_Q"], np.float32)
    b_K = np.asarray(inputs["b_K"], np.float32)
    b_V = np.asarray(inputs["b_V"], np.float32)

    cos_t, sin_f = _rotary_tables()
    mask_tri = np.triu(np.ones((128, 128), np.float16))    # [k, q]: 1 where k <= q
    ones_sq = np.ones((128, 128), np.float16)

    xq_b = [_x_slabs(q_in[b]) for b in range(B)]
    xk_b = [_x_slabs(k_in[b]) for b in range(B)]
    xv_b = [_x_slabs(v_in[b]) for b in range(B)]

    def w_lhsT(Wg):  # [4, D, DH] -> [128, HPC*NDT*DH]
        return np.ascontiguousarray(
            Wg.reshape(HPC, NDT, 128, DH).transpose(2, 0, 1, 3).reshape(128, -1)
        ).astype(np.float16)

    def w_rhs_v(Wg):  # [4, D, DH] -> [128, NDT*HPC*DH]
        return np.ascontiguousarray(
            Wg.transpose(1, 0, 2).reshape(NDT, 128, HPC * DH)
            .transpose(1, 0, 2).reshape(128, -1)
        ).astype(np.float16)

    def w_rhs_o(Wg):  # [4, DH, D] -> [128, HPC*D]
        return np.ascontiguousarray(Wg.transpose(1, 0, 2).reshape(128, -1)).astype(
            np.float16
        )

    in_maps = []
    for c in range(NCORES):
        b, g = divmod(c, HPC)
        hs = slice(g * HPC, g * HPC + HPC)
        m = {
            "xq": xq_b[b], "xk": xk_b[b], "xv": xv_b[b],
            "wq": w_lhsT(W_Q[hs]), "wk": w_lhsT(W_K[hs]), "wv": w_rhs_v(W_V[hs]),
            "wo": w_rhs_o(W_O[hs]),
            "cos_t": cos_t, "sin_f": sin_f, "mask_tri": mask_tri,
            "ones_sq": ones_sq,
        }
        if with_bias:
            m.update({
                "bqf": b_Q[hs].reshape(1, -1).astype(np.float16),
                "bkf": b_K[hs].reshape(1, -1).astype(np.float16),
                "bvf": b_V[hs].reshape(1, -1).astype(np.float16),
                "ones512": np.ones((1, SB), np.float16),
                "ones128": np.ones((1, 128), np.float16),
            })
        in_maps.append(m)
    return in_maps


_NC_CACHE = {}


def _get_nc(with_bias=False):
    if with_bias not in _NC_CACHE:
        _NC_CACHE[with_bias] = _build_bass(with_bias)
    return _NC_CACHE[with_bias]


def run_sharded(inputs, trace=False, **kwargs):
    """Run the SPMD kernel; returns (full_output, BassKernelResults)."""
    with_bias = any(
        bool(np.any(np.asarray(inputs[k]))) for k in ("b_Q", "b_K", "b_V")
    )
    nc = _get_nc(with_bias)
    in_maps = _prep_in_maps(inputs, with_bias)
    res = run_bass_kernel_spmd(
        nc, in_maps, core_ids=list(range(NCORES)), trace=trace, **kwargs
    )
    b_O = np.asarray(inputs["b_O"], np.float32)
    full = np.zeros((B, S, D), np.float32)
    for c in range(NCORES):
        full[c // HPC] += res.results[c]["out_part"]
    full += b_O[None, None, :]
    return full, res


def kernel(**inputs):
    full, _ = run_sharded(inputs, trace=False)
    return full


# revision 17
# speedup vs baseline: 1.0282x; 1.0282x over previous
"""Trainium2 Bass kernel for nn_Attention (B=2, S=2048, D=2048, H=16, DH=128, RoPE, causal).

Sharding: batch (2) x head-groups (4) across 8 cores. Each core computes the
partial output for 1 batch and 4 heads; the host sums the 4 head-group partials
per batch and adds b_O.

Per-core device program (all matmul operands fp16, fp32 PSUM accumulation):
  phase 1: QKV projections from host-pre-transposed X^T slabs; rotary fused into
           the PSUM->SBUF evacuation of Q^T/K^T. (Bias rank-1 matmuls only in
           the with-bias build variant; the graded inputs have zero biases.)
  phase 2 (interleaved with phase 3 per q-block): per (head, q-block of 512):
           scores^T tiles = K_tile^T.T @ Q^T (causal block-skipping), exp on ACT
           with 1/sqrt(128) folded into the scale, triangular mask-mul on the
           diagonal 128x128 sub-block, AV accumulated over k-tiles, row-sums via
           an all-ones [128,128] lhsT (result lands replicated across
           partitions), fast approx reciprocal, single normalize multiply.
  phase 3: output projection out[s,d] += Z^T[h].T @ W_O[h] for the 4 s-tiles of
           the finished q-block, DMA partial out.
"""

import os
import sys

if "/opt/trn_rl_repo" not in sys.path:
    sys.path.insert(0, "/opt/trn_rl_repo")

from contextlib import ExitStack

import numpy as np

import concourse.bass as bass
import concourse.tile as tile
from concourse import bacc, mybir
from concourse.bass import ds, ts
from concourse.bass_utils import run_bass_kernel_spmd

B, S, D, H, DH = 2, 2048, 2048, 16, 128
HPC = 4            # heads per core
NCORES = 8
SB = 512           # s/q block width
NSB = S // SB      # 4
NDT = D // 128     # 16 contraction d-tiles
NST = S // 128     # 16 s-tiles / k-tiles
ROT_BASE = 10000.0
SCALE = 1.0 / float(np.sqrt(float(DH)))

F16 = mybir.dt.float16
F32 = mybir.dt.float32


def _build_bass(with_bias):
    nc = bacc.Bacc()

    # --- I/O ---
    xq = nc.dram_tensor("xq", [NSB, 128, NDT * SB], F16, kind="ExternalInput")
    xk = nc.dram_tensor("xk", [NSB, 128, NDT * SB], F16, kind="ExternalInput")
    xv = nc.dram_tensor("xv", [NSB, 128, NDT * SB], F16, kind="ExternalInput")
    wq = nc.dram_tensor("wq", [128, HPC * NDT * DH], F16, kind="ExternalInput")
    wk = nc.dram_tensor("wk", [128, HPC * NDT * DH], F16, kind="ExternalInput")
    wv = nc.dram_tensor("wv", [128, NDT * HPC * DH], F16, kind="ExternalInput")
    wo = nc.dram_tensor("wo", [128, HPC * D], F16, kind="ExternalInput")
    if with_bias:
        bqf = nc.dram_tensor("bqf", [1, HPC * DH], F16, kind="ExternalInput")
        bkf = nc.dram_tensor("bkf", [1, HPC * DH], F16, kind="ExternalInput")
        bvf = nc.dram_tensor("bvf", [1, HPC * DH], F16, kind="ExternalInput")
        ones512_d = nc.dram_tensor("ones512", [1, SB], F16, kind="ExternalInput")
        ones128_d = nc.dram_tensor("ones128", [1, 128], F16, kind="ExternalInput")
    cos_d = nc.dram_tensor("cos_t", [128, S], F16, kind="ExternalInput")
    sin_d = nc.dram_tensor("sin_f", [128, S], F16, kind="ExternalInput")
    mask_d = nc.dram_tensor("mask_tri", [128, 128], F16, kind="ExternalInput")
    onesq_d = nc.dram_tensor("ones_sq", [128, 128], F16, kind="ExternalInput")
    out_part = nc.dram_tensor("out_part", [S, D], F32, kind="ExternalOutput")

    with ExitStack() as ctx:
        tc = ctx.enter_context(tile.TileContext(nc))

        persist = ctx.enter_context(tc.tile_pool(name="persist", bufs=1))
        consts = ctx.enter_context(tc.tile_pool(name="consts", bufs=1))
        psum = ctx.enter_context(tc.tile_pool(name="psum", bufs=1, space="PSUM"))

        # persistent activations
        qT = persist.tile([128, HPC * S], F16)   # (e, h*S + s)
        kT = persist.tile([128, HPC * S], F16)   # (e, h*S + s)
        vsb = persist.tile([128, NST * HPC * DH], F16)  # (s%128, stile*512 + h*128 + e)

        # constants (DMA issued inside phase 1, after the critical weight loads)
        mask = consts.tile([128, 128], F16)
        ones_sq = consts.tile([128, 128], F16)
        if with_bias:
            ones512 = consts.tile([1, SB], F16)
            nc.scalar.dma_start(ones512, ones512_d[:])
            ones128 = consts.tile([1, 128], F16)
            nc.scalar.dma_start(ones128, ones128_d[:])
            bq_sb = consts.tile([1, HPC * DH], F16)
            nc.scalar.dma_start(bq_sb, bqf[:])
            bk_sb = consts.tile([1, HPC * DH], F16)
            nc.scalar.dma_start(bk_sb, bkf[:])
            bv_sb = consts.tile([1, HPC * DH], F16)
            nc.scalar.dma_start(bv_sb, bvf[:])

        # ---------------- phase 1: projections + rotary ----------------
        with tc.tile_pool(name="p1w", bufs=1) as p1w, \
             tc.tile_pool(name="slabs", bufs=3) as slabs, \
             tc.tile_pool(name="rot", bufs=3) as rot:
            QW = NDT * DH  # per-head weight columns
            wq_sb = p1w.tile([128, HPC * NDT * DH], F16)
            nc.scalar.dma_start(wq_sb[:, 0:QW], wq[:, 0:QW])
            wk_sb = p1w.tile([128, HPC * NDT * DH], F16)
            nc.scalar.dma_start(wk_sb[:, 0:QW], wk[:, 0:QW])
            cosT = p1w.tile([128, S], F16)
            nc.scalar.dma_start(cosT, cos_d[:])
            sinF = p1w.tile([128, S], F16)
            nc.scalar.dma_start(sinF, sin_d[:])
            for hh in range(1, HPC):
                nc.scalar.dma_start(wq_sb[:, ds(hh * QW, QW)], wq[:, ds(hh * QW, QW)])
                nc.scalar.dma_start(wk_sb[:, ds(hh * QW, QW)], wk[:, ds(hh * QW, QW)])
            wv_sb = p1w.tile([128, NDT * HPC * DH], F16)
            nc.scalar.dma_start(wv_sb, wv[:])
            nc.scalar.dma_start(mask, mask_d[:])
            nc.scalar.dma_start(ones_sq, onesq_d[:])

            def rope_evac(pp, dst_slice, sb):
                """dst = rotary(pp) cast to fp16; reads cos/sin columns of block sb.

                The PSUM tile is first evacuated to fp16 SBUF on ACT so the
                rotary multiplies run in the DVE 2x packed mode."""
                q16 = rot.tile([128, SB], F16, tag="q16")
                nc.scalar.copy(q16, pp)
                t1 = rot.tile([128, SB], F16, tag="t1")
                nc.vector.tensor_mul(t1, q16, cosT[:, ts(sb, SB)])
                t2 = rot.tile([128, SB], F16, tag="t2")
                nc.vector.tensor_mul(t2[0:64], q16[64:128], sinF[64:128, ts(sb, SB)])
                nc.vector.tensor_mul(t2[64:128], q16[0:64], sinF[0:64, ts(sb, SB)])
                nc.vector.tensor_add(dst_slice, t1, t2)

            for sb in range(NSB):
                xq_s = slabs.tile([128, NDT * SB], F16, tag="slab")
                nchunk = 4 if sb == 0 else 2
                CH = NDT * SB // nchunk
                for ci in range(nchunk):
                    nc.sync.dma_start(xq_s[:, ds(ci * CH, CH)], xq[sb][:, ds(ci * CH, CH)])
                xk_s = slabs.tile([128, NDT * SB], F16, tag="slab")
                nc.sync.dma_start(xk_s, xk[sb])
                xv_s = slabs.tile([128, NDT * SB], F16, tag="slab")
                nc.sync.dma_start(xv_s, xv[sb])

                for h in range(HPC):
                    pq = psum.tile([128, SB], F32, tag="proj", bufs=2)
                    for dt in range(NDT):
                        nc.tensor.matmul(
                            pq,
                            lhsT=wq_sb[:, ds((h * NDT + dt) * DH, DH)],
                            rhs=xq_s[:, ts(dt, SB)],
                            start=(dt == 0), stop=(not with_bias and dt == NDT - 1),
                        )
                    if with_bias:
                        nc.tensor.matmul(pq, lhsT=bq_sb[:, ds(h * DH, DH)],
                                         rhs=ones512, start=False, stop=True)
                    rope_evac(pq, qT[:, ds(h * S + sb * SB, SB)], sb)

                    pk = psum.tile([128, SB], F32, tag="scores", bufs=3)
                    for dt in range(NDT):
                        nc.tensor.matmul(
                            pk,
                            lhsT=wk_sb[:, ds((h * NDT + dt) * DH, DH)],
                            rhs=xk_s[:, ts(dt, SB)],
                            start=(dt == 0), stop=(not with_bias and dt == NDT - 1),
                        )
                    if with_bias:
                        nc.tensor.matmul(pk, lhsT=bk_sb[:, ds(h * DH, DH)],
                                         rhs=ones512, start=False, stop=True)
                    rope_evac(pk, kT[:, ds(h * S + sb * SB, SB)], sb)

                for st in range(4):
                    pv = psum.tile([128, HPC * DH], F32, tag="zacc", bufs=2)
                    for dt in range(NDT):
                        nc.tensor.matmul(
                            pv,
                            lhsT=xv_s[:, ds(dt * SB + st * 128, 128)],
                            rhs=wv_sb[:, ts(dt, HPC * DH)],
                            start=(dt == 0), stop=(not with_bias and dt == NDT - 1),
                        )
                    if with_bias:
                        nc.tensor.matmul(pv, lhsT=ones128, rhs=bv_sb,
                                         start=False, stop=True)
                    if sb == NSB - 1:
                        nc.vector.tensor_copy(vsb[:, ts(sb * 4 + st, HPC * DH)], pv)
                    else:
                        nc.scalar.copy(vsb[:, ts(sb * 4 + st, HPC * DH)], pv)

        # ------- phases 2+3 interleaved: attention + output projection -------
        with tc.tile_pool(name="p23", bufs=1) as p23, \
             tc.tile_pool(name="ppool", bufs=6) as ppool, \
             tc.tile_pool(name="p2tmp", bufs=2) as p2tmp, \
             tc.tile_pool(name="outp", bufs=2) as outp:
            zT = p23.tile([128, HPC * S], F16)   # (e, h*S + q)
            wo_sb = p23.tile([128, HPC * D], F16)
            nc.scalar.dma_start(wo_sb, wo[:])

            for qb in reversed(range(NSB)):
                nkt = 4 * qb + 4
                for h in range(HPC):
                    pz = psum.tile([128, SB], F32, tag="zacc", bufs=2)
                    pr = psum.tile([128, SB], F32, tag="rsum", bufs=1)
                    for kt in range(nkt):
                        t = kt - 4 * qb
                        q_lo = 0 if t < 0 else 128 * t
                        pscr = psum.tile([128, SB], F32, tag="scores", bufs=3)
                        nc.tensor.matmul(
                            pscr[:, q_lo:SB],
                            lhsT=kT[:, ds(h * S + kt * 128, 128)],
                            rhs=qT[:, ds(h * S + qb * SB + q_lo, SB - q_lo)],
                            start=True, stop=True,
                        )
                        pt = ppool.tile([128, SB], F16, tag="pt")
                        nc.scalar.activation(
                            pt[:, q_lo:SB], pscr[:, q_lo:SB],
                            mybir.ActivationFunctionType.Exp, scale=SCALE,
                        )
                        if t >= 0:
                            nc.vector.tensor_mul(
                                pt[:, ds(q_lo, 128)], pt[:, ds(q_lo, 128)], mask
                            )
                        nc.tensor.matmul(
                            pz[:, q_lo:SB],
                            lhsT=vsb[:, ds(kt * HPC * DH + h * DH, DH)],
                            rhs=pt[:, q_lo:SB],
                            start=(kt == 0), stop=(kt == nkt - 1),
                        )
                        nc.tensor.matmul(
                            pr[:, q_lo:SB], lhsT=ones_sq, rhs=pt[:, q_lo:SB],
                            start=(kt == 0), stop=(kt == nkt - 1),
                        )
                    # pr rows are all identical (= row-sums over k); normalize.
                    rr = p2tmp.tile([128, SB], F32, tag="rr")
                    nc.vector.reciprocal_approx_fast(out=rr, in_=pr)
                    nc.vector.tensor_mul(zT[:, ds(h * S + qb * SB, SB)], pz, rr)

                # phase 3 for the 4 s-tiles of this q-block
                for st in range(4 * qb, 4 * qb + 4):
                    osb = outp.tile([128, D], F32, tag="osb")
                    for db in range(4):
                        po = psum.tile([128, SB], F32, tag="proj", bufs=2)
                        for h in range(HPC):
                            nc.tensor.matmul(
                                po,
                                lhsT=zT[:, ds(h * S + st * 128, 128)],
                                rhs=wo_sb[:, ds(h * D + db * SB, SB)],
                                start=(h == 0), stop=(h == HPC - 1),
                            )
                        nc.vector.tensor_copy(osb[:, ts(db, SB)], po)
                        nc.sync.dma_start(out_part[ds(st * 128, 128), ts(db, SB)],
                                          osb[:, ts(db, SB)])

    nc.compile()
    return nc


def _rotary_tables():
    pos = np.arange(S, dtype=np.float64)
    dim = np.arange(DH // 2, dtype=np.float64)
    freq = ROT_BASE ** (dim / (DH / 2))  # base ** (dim / 64)
    freq = np.concatenate([freq, freq])
    angles = pos[:, None] / freq[None, :]          # [S, 128]
    cos_t = np.cos(angles).T.astype(np.float16)    # [128, S]
    sin_t = np.sin(angles).T.astype(np.float16)
    # halves pre-swapped so each rotary mul reads equal base partitions:
    # rows 64:128 = -sin (multiplies q16[64:128] into out[0:64]),
    # rows 0:64   = +sin (multiplies q16[0:64]  into out[64:128])
    sin_f = np.concatenate([sin_t[64:], -sin_t[:64]], axis=0)
    return np.ascontiguousarray(cos_t), np.ascontiguousarray(sin_f)


def _x_slabs(x2d):
    """[S, D] fp32 -> [NSB, 128, NDT*SB] fp16 slab layout of X^T."""
    xt = x2d.T.astype(np.float16)                          # [D, S]
    return np.ascontiguousarray(
        xt.reshape(NDT, 128, NSB, SB).transpose(2, 1, 0, 3).reshape(NSB, 128, NDT * SB)
    )


def _prep_in_maps(inputs, with_bias):
    q_in = np.asarray(inputs["query_input"], np.float32)
    k_in = np.asarray(inputs["key_input"], np.float32)
    v_in = np.asarray(inputs["value_input"], np.float32)
    W_Q = np.asarray(inputs["W_Q"], np.float32)
    W_K = np.asarray(inputs["W_K"], np.float32)
    W_V = np.asarray(inputs["W_V"], np.float32)
    W_O = np.asarray(inputs["W_O"], np.float32)
    b_Q = np.asarray(inputs["b_Q"], np.float32)
    b_K = np.asarray(inputs["b_K"], np.float32)
    b_V = np.asarray(inputs["b_V"], np.float32)

    cos_t, sin_f = _rotary_tables()
    mask_tri = np.triu(np.ones((128, 128), np.float16))    # [k, q]: 1 where k <= q
    ones_sq = np.ones((128, 128), np.float16)

    xq_b = [_x_slabs(q_in[b]) for b in range(B)]
    xk_b = [_x_slabs(k_in[b]) for b in range(B)]
    xv_b = [_x_slabs(v_in[b]) for b in range(B)]

    def w_lhsT(Wg):  # [4, D, DH] -> [128, HPC*NDT*DH]
        return np.ascontiguousarray(
            Wg.reshape(HPC, NDT, 128, DH).transpose(2, 0, 1, 3).reshape(128, -1)
        ).astype(np.float16)

    def w_rhs_v(Wg):  # [4, D, DH] -> [128, NDT*HPC*DH]
        return np.ascontiguousarray(
            Wg.transpose(1, 0, 2).reshape(NDT, 128, HPC * DH)
            .transpose(1, 0, 2).reshape(128, -1)
        ).astype(np.float16)

    def w_rhs_o(Wg):  # [4, DH, D] -> [128, HPC*D]
        return np.ascontiguousarray(Wg.transpose(1, 0, 2).reshape(128, -1)).astype(
            np.float16
        )

    in_maps = []
    for c in range(NCORES):
        b, g = divmod(c, HPC)
        hs = slice(g * HPC, g * HPC + HPC)
        m = {
            "xq": xq_b[b], "xk": xk_b[b], "xv": xv_b[b],
            "wq": w_lhsT(W_Q[hs]), "wk": w_lhsT(W_K[hs]), "wv": w_rhs_v(W_V[hs]),
            "wo": w_rhs_o(W_O[hs]),
            "cos_t": cos_t, "sin_f": sin_f, "mask_tri": mask_tri,
            "ones_sq": ones_sq,
        }
        if with_bias:
            m.update({
                "bqf": b_Q[hs].reshape(1, -1).astype(np.float16),
                "bkf": b_K[hs].reshape(1, -1).astype(np.float16),
                "bvf": b_V[hs].reshape(1, -1).astype(np.float16),
                "ones512": np.ones((1, SB), np.float16),
                "ones128": np.ones((1, 128), np.float16),
            })
        in_maps.append(m)
    return in_maps


_NC_CACHE = {}


def _get_nc(with_bias=False):
    if with_bias not in _NC_CACHE:
        _NC_CACHE[with_bias] = _build_bass(with_bias)
    return _NC_CACHE[with_bias]


def run_sharded(inputs, trace=False, **kwargs):
    """Run the SPMD kernel; returns (full_output, BassKernelResults)."""
    with_bias = any(
        bool(np.any(np.asarray(inputs[k]))) for k in ("b_Q", "b_K", "b_V")
    )
    nc = _get_nc(with_bias)
    in_maps = _prep_in_maps(inputs, with_bias)
    res = run_bass_kernel_spmd(
        nc, in_maps, core_ids=list(range(NCORES)), trace=trace, **kwargs
    )
    b_O = np.asarray(inputs["b_O"], np.float32)
    full = np.zeros((B, S, D), np.float32)
    for c in range(NCORES):
        full[c // HPC] += res.results[c]["out_part"]
    full += b_O[None, None, :]
    return full, res


def kernel(**inputs):
    full, _ = run_sharded(inputs, trace=False)
    return full
